# revision 2
# baseline (speedup 1.0000x reference)
"""Trainium2 Bass kernel for nn_CauseAttention (dense transformer block).

Reference computation (per batch b):
    qkv = x @ W_attn + b_attn          # [T, 3N]
    q, k, v  (heads H=16, HD=64)
    att = softmax(q k^T / sqrt(T))     # NOTE scale 1/sqrt(T) = 1/32, no mask
    y   = att @ v                      # [T, N]
    out = y @ W_proj + b_proj

Sharding: data-parallel over batch. B=16 batches across 8 NeuronCores,
2 batches per core. No collectives.

Per-core pipeline (fp8e4m3 on the q/k path where softmax damps the
quantization error ~10x; bf16 everywhere that feeds the output directly):

  - x^T via PE transpose (bf16), then xT8 = fp8(32*xT).
  - W_attn q/k columns loaded straight to fp8 (DMA cast); v columns bf16.
  - qk^T GEMM in fp8 DoubleRow perf mode: contraction pairs are adjacent
    kt-tiles ([128, 2, *] APs), 4 chained DoubleRow matmuls instead of 8
    bf16 ones -> 4x fewer PE cycles.  Output q8/k8 = 32*(q|k)+32*b in fp8.
  - scores S^T per (head, kt): DoubleRow with BOTH operands 0-stride
    broadcast over the pair dim -> computes 2*(k8^T q8) at 0.5 cycles/col
    (the doubling is folded into the exp scale 2^-16).  Contraction is the
    64-row head dim; cost halves vs bf16 despite K=64.
  - exp on ScalarE straight from PSUM (logits tiny, no max-subtraction),
    bf16 att tiles.
  - y = att @ v in NATURAL layout: lhsT = attT q-slice, rhs = [v|ones]
    (65 columns streamed instead of 1024 -> ~2x fewer PE cycles than the
    transposed formulation).  Row sums land in column 64; normalization is
    a DVE reciprocal + per-partition tensor_scalar multiply.
  - y^T via PE transpose, then out = y^T.T @ W_proj + b_proj (bf16).

Emission interleaving: the softmax exp stream (ScalarE ~118us/batch) gates
the attention phase, so PE filler work (next batch's transposes/qk GEMM/v
GEMM, previous batch's projection) is emitted between the per-kt score
matmuls to keep the in-order PE queue fed inside the Act-bound window.
"""

import sys

for _p in ("/opt/trn_rl_repo", "/opt/pypackages"):
    if _p not in sys.path:
        sys.path.append(_p)

from collections import deque

import numpy as np

import concourse.bass as bass
import concourse.mybir as mybir
import concourse.tile as tile
from concourse import bacc
from concourse.masks import make_identity

F32 = mybir.dt.float32
BF16 = mybir.dt.bfloat16
FP8 = mybir.dt.float8e4
AF = mybir.ActivationFunctionType
ALU = mybir.AluOpType
PM = mybir.MatmulPerfMode

# Problem shapes (hardcoded per spec)
B, T, N, H = 16, 1024, 1024, 16
HD = N // H              # 64
NCORES = 8
BL = B // NCORES         # batches per core = 2
P = 128
KT = N // P              # 8 k-tiles of the model dim
NT = T // P              # 8 token tiles per batch
NF = (2 * N) // P        # 16 qk output tiles
H2 = H // 2              # 8 head pairs
CH = 2
C512 = 512
SX = 32.0                # x scale before fp8 quantization (weights unscaled)
# exp(EXP_SCALE * S2) where S2 = 2 * (SX*q)·(SX*k) = 2*SX^2 * (q·k);
# reference scale is 1/sqrt(T) = 1/32 -> EXP_SCALE = (1/32)/(2*SX^2) = 2^-16
EXP_SCALE = (1.0 / 32.0) / (2.0 * SX * SX)


def _build_nc() -> bass.Bass:
    nc = bacc.Bacc("TRN2", target_bir_lowering=False, debug=False, num_devices=NCORES)

    x = nc.dram_tensor("x", [BL * T, N], F32, kind="ExternalInput").ap()
    wa = nc.dram_tensor("W_attn", [N, 3 * N], F32, kind="ExternalInput").ap()
    ba = nc.dram_tensor("b_attn", [3 * N], F32, kind="ExternalInput").ap()
    wp = nc.dram_tensor("W_proj", [N, N], F32, kind="ExternalInput").ap()
    bp = nc.dram_tensor("b_proj", [N], F32, kind="ExternalInput").ap()
    out = nc.dram_tensor("out", [BL * T, N], F32, kind="ExternalOutput").ap()

    wa_r = wa.rearrange("(kt p) n -> p kt n", p=P)

    with tile.TileContext(nc) as tc:
        with (
            tc.tile_pool(name="wpool", bufs=1) as wpool,
            tc.tile_pool(name="bpool", bufs=1) as bpool,
            tc.tile_pool(name="apool", bufs=2) as apool,
            tc.tile_pool(name="npool", bufs=1) as npool,
            tc.tile_pool(name="opool", bufs=3) as opool,
            tc.tile_pool(name="pss", bufs=2, space="PSUM") as pss_pool,
            tc.tile_pool(name="psm", bufs=2, space="PSUM") as psm_pool,
            tc.tile_pool(name="psy", bufs=2, space="PSUM") as psy_pool,
        ):
            identity = wpool.tile([P, P], BF16, name="identity")
            make_identity(nc, identity)

            # ---------------- persistent weight / bias tiles ----------------
            # (loads emitted lazily below so batch-0 x transposes go first)
            wa8 = wpool.tile([P, KT, 2 * N], FP8, name="wa8")
            wav = wpool.tile([P, KT, N], BF16, name="wav")
            wp_sb = wpool.tile([P, KT, N], BF16, name="wp_sb")
            bqk32 = wpool.tile([P, NF], F32, name="bqk32")
            ones_row = wpool.tile([P, P], BF16, name="ones_row")
            bv_bc = wpool.tile([P, N], BF16, name="bv_bc")
            bp_bc = wpool.tile([P, N], BF16, name="bp_bc")

            def emit_weight_loads():
                bqk_raw = npool.tile([P, NF], F32, name="bqk_raw", tag="bqk_raw")
                nc.sync.dma_start(
                    bqk_raw[:], ba[0 : 2 * N].rearrange("(o p) -> p o", p=P)
                )
                nc.vector.tensor_scalar_mul(bqk32[:], bqk_raw[:], SX)
                # q/k weight columns: straight f32 -> fp8 cast loads (chunked)
                for c0 in range(0, 2 * N, N):
                    nc.gpsimd.dma_start(
                        wa8[:, :, c0 : c0 + N], wa_r[:, :, c0 : c0 + N]
                    )
                nc.gpsimd.dma_start(wav[:], wa_r[:, :, 2 * N : 3 * N])
                nc.vector.memset(ones_row[:], 1.0)
                bv_row = npool.tile([1, N], BF16, name="bv_row", tag="brow", bufs=2)
                nc.gpsimd.dma_start(
                    bv_row[:], ba[2 * N : 3 * N].rearrange("(a n) -> a n", a=1)
                )
                bp_row = npool.tile([1, N], BF16, name="bp_row", tag="brow", bufs=2)
                nc.gpsimd.dma_start(bp_row[:], bp.rearrange("(a n) -> a n", a=1))
                for row, bc in ((bv_row, bv_bc), (bp_row, bp_bc)):
                    for c in range(CH):
                        cs = slice(c * C512, (c + 1) * C512)
                        pst = psm_pool.tile([P, C512], F32, tag="mm", name="ps_bcast")
                        nc.tensor.matmul(
                            pst[:], ones_row[0:1, :], row[:, cs], start=True, stop=True
                        )
                        nc.vector.tensor_copy(bc[:, cs], pst[:])
                nc.gpsimd.dma_start(wp_sb[:], wp.rearrange("(kt p) n -> p kt n", p=P))

            # ---------------- per-batch big tiles ----------------
            def mk_q8(b):
                return bpool.tile([P, H2, T], FP8, name=f"q8_{b}", tag="q8")

            def mk_k8(b):
                return bpool.tile([P, H2, KT, P], FP8, name=f"k8_{b}", tag="k8")

            # ---------------- x load + transpose + fp8 cast ----------------
            def emit_x_loads(b):
                xs = []
                x_r = x[b * T : (b + 1) * T, :].rearrange("(tt p) n -> p tt n", p=P)
                for tt in range(NT):
                    x_sb = bpool.tile(
                        [P, N], BF16, name=f"x_sb{b}_{tt}", tag="x_sb", bufs=2
                    )
                    nc.gpsimd.dma_start(x_sb[:], x_r[:, tt, :])
                    xs.append(x_sb)
                return xs

            def mk_xT(b):
                xT = bpool.tile([P, KT, T], BF16, name=f"xT_{b}", tag="xT")
                xT8 = bpool.tile([P, KT, T], FP8, name=f"xT8_{b}", tag="xT8")
                return xT, xT8

            def emit_xT_tt(b, xs, xT, tt):
                # transpose all 8 feature blocks of token-tile tt
                for kf in range(KT):
                    if kf % 2 == 0:
                        pst = psm_pool.tile([P, P], BF16, tag="mm", name="ps_tr")
                    else:
                        pst = psy_pool.tile(
                            [P, P], BF16, tag="y", name="ps_tr2",
                            padded_shape=[P, 1024],
                        )
                    nc.tensor.transpose(
                        pst[:], xs[tt][:, kf * P : (kf + 1) * P], identity[:]
                    )
                    nc.vector.tensor_copy(xT[:, kf, tt * P : (tt + 1) * P], pst[:])

            def emit_xT8_kf(b, xT, xT8, kf):
                nc.vector.tensor_scalar_mul(xT8[:, kf, :], xT[:, kf, :], SX)

            # ---------------- qk GEMM (fp8 DoubleRow, kt-paired) -----------
            def emit_qk_nf(b, xT8, q8, k8, nf):
                for cc in range(CH):
                    cs = slice(cc * C512, (cc + 1) * C512)
                    pst = psm_pool.tile([P, C512], F32, tag="mm", name="ps_qk")
                    for c in range(4):
                        nc.tensor.matmul(
                            pst[:],
                            wa8[:, 2 * c : 2 * c + 2, nf * P : (nf + 1) * P],
                            xT8[:, 2 * c : 2 * c + 2, cs],
                            start=(c == 0),
                            stop=(c == 3),
                            perf_mode=PM.DoubleRow,
                        )
                    if nf < H2:
                        nc.vector.tensor_scalar_add(
                            q8[:, nf, cs], pst[:], bqk32[:, nf : nf + 1]
                        )
                    else:
                        nc.vector.tensor_scalar_add(
                            k8[:, nf - H2, 4 * cc : 4 * cc + 4, :],
                            pst[:].rearrange("p (kt j) -> p kt j", j=P),
                            bqk32[:, nf : nf + 1],
                        )

            # ---------------- v GEMM (bf16, natural layout) ----------------
            def mk_v(b):
                v_sb = bpool.tile(
                    [P, NT, H, 65], BF16, name=f"v_sb{b}", tag="v_sb", bufs=2
                )
                nc.vector.memset(v_sb[:, :, :, 64:65], 1.0)
                return v_sb

            def emit_v_tt(b, xT, v_sb, tt, cc):
                cs = slice(cc * C512, (cc + 1) * C512)
                pst = psm_pool.tile([P, C512], F32, tag="mm", name="ps_v")
                for kt in range(KT):
                    nc.tensor.matmul(
                        pst[:],
                        xT[:, kt, tt * P : (tt + 1) * P],
                        wav[:, kt, cs],
                        start=(kt == 0),
                        stop=(kt == KT - 1),
                    )
                hh = slice(cc * 8, cc * 8 + 8)
                nc.vector.tensor_tensor(
                    v_sb[:, tt, hh, 0:64],
                    pst[:].rearrange("p (h d) -> p h d", d=HD),
                    bv_bc[:, cs].rearrange("p (h d) -> p h d", d=HD),
                    ALU.add,
                )

            # ---------------- attention ----------------
            def emit_S_head(b, q8, k8, h, fillers, cur_h):
                """score matmuls + exp for head h; pops ready fillers
                between kt tiles to keep the PE queue fed."""
                base = (h % 2) * 64
                hp = h // 2
                att_kt = []
                for kt in range(KT):
                    pst = pss_pool.tile([P, T], F32, tag="s", name="ps_s")
                    lhs = (
                        k8[base : base + 64, hp, kt, :]
                        .unsqueeze(1)
                        .broadcast_to([64, 2, P])
                    )
                    for cc in range(CH):
                        cs = slice(cc * C512, (cc + 1) * C512)
                        rhs = (
                            q8[base : base + 64, hp, cs]
                            .unsqueeze(1)
                            .broadcast_to([64, 2, C512])
                        )
                        nc.tensor.matmul(
                            pst[:, cs], lhs, rhs,
                            start=True, stop=True, perf_mode=PM.DoubleRow,
                        )
                    attT = apool.tile([P, T], BF16, name="attT", tag="attT", bufs=16)
                    nc.scalar.activation(attT[:], pst[:], AF.Exp, scale=EXP_SCALE)
                    att_kt.append(attT)
                    # pop at most one ready filler per kt slot
                    if fillers and fillers[0][0] <= cur_h:
                        fillers.popleft()[1]()
                return att_kt

            def emit_y_head(b, att_kt, v_sb, y_sb, h):
                for qt in range(NT):
                    psy = psy_pool.tile(
                        [P, 65], F32, tag="y", name="ps_y", padded_shape=[P, C512]
                    )
                    for kt in range(KT):
                        nc.tensor.matmul(
                            psy[:],
                            att_kt[kt][:, qt * P : (qt + 1) * P],
                            v_sb[:, kt, h, :],
                            start=(kt == 0),
                            stop=(kt == KT - 1),
                        )
                    rf = npool.tile([P, 1], F32, name="rf", tag="rf", bufs=4)
                    nc.vector.reciprocal_approx_fast(rf[:], psy[:, 64:65])
                    nc.vector.tensor_scalar_mul(
                        y_sb[:, qt, h * HD : (h + 1) * HD], psy[:, 0:64], rf[:, 0:1]
                    )

            # ---------------- y transpose + projection ----------------
            def emit_yT_qt(b, y_sb, yT, qt):
                for kf in range(KT):
                    if kf % 2 == 0:
                        pst = psm_pool.tile([P, P], BF16, tag="mm", name="ps_ytr")
                    else:
                        pst = psy_pool.tile(
                            [P, P], BF16, tag="y", name="ps_ytr2",
                            padded_shape=[P, 1024],
                        )
                    nc.tensor.transpose(
                        pst[:], y_sb[:, qt, kf * P : (kf + 1) * P], identity[:]
                    )
                    nc.vector.tensor_copy(yT[:, kf, qt * P : (qt + 1) * P], pst[:])

            def emit_proj_tt(b, yT, tt, cc):
                cs = slice(cc * C512, (cc + 1) * C512)
                pst = psm_pool.tile([P, C512], F32, tag="mm", name="ps_o")
                for kt in range(KT):
                    nc.tensor.matmul(
                        pst[:],
                        yT[:, kt, tt * P : (tt + 1) * P],
                        wp_sb[:, kt, cs],
                        start=(kt == 0),
                        stop=(kt == KT - 1),
                    )
                osb = opool.tile([P, C512], F32, name="osb", tag="osb", bufs=2)
                nc.vector.tensor_tensor(osb[:], pst[:], bp_bc[:, cs], ALU.add)
                nc.sync.dma_start(
                    out[b * T + tt * P : b * T + (tt + 1) * P, cs], osb[:]
                )

            # ================ emission schedule ================
            # prologue: batch 0 x pipeline + weights + qk(0) + v(0)
            xs0 = emit_x_loads(0)
            emit_weight_loads()
            xT0, xT80 = mk_xT(0)
            for tt in range(NT):
                emit_xT_tt(0, xs0, xT0, tt)
            for kf in range(KT):
                emit_xT8_kf(0, xT0, xT80, kf)
            q8_0, k8_0 = mk_q8(0), mk_k8(0)
            for nf in range(NF):
                emit_qk_nf(0, xT80, q8_0, k8_0, nf)
            v0 = mk_v(0)
            for tt in range(NT):
                for cc in range(CH):
                    emit_v_tt(0, xT0, v0, tt, cc)

            # fillers for heads(0): batch-1 x pipeline, qk(1), v(1)
            xT1, xT81 = mk_xT(1)
            q8_1, k8_1 = mk_q8(1), mk_k8(1)
            v1 = mk_v(1)
            xs1 = emit_x_loads(1)
            f0: deque = deque()
            for tt in range(NT):
                f0.append((tt // 4, lambda tt=tt: emit_xT_tt(1, xs1, xT1, tt)))
            for kf in range(KT):
                f0.append((2, lambda kf=kf: emit_xT8_kf(1, xT1, xT81, kf)))
            # qk(1, nf) gated until batch-0 head pair nf's scores are done
            for nfq in range(H2):
                f0.append(
                    (2 * nfq + 2, lambda nf=nfq: emit_qk_nf(1, xT81, q8_1, k8_1, nf))
                )
                f0.append(
                    (2 * nfq + 2,
                     lambda nf=nfq + H2: emit_qk_nf(1, xT81, q8_1, k8_1, nf))
                )
            for tt in range(NT):
                for cc in range(CH):
                    f0.append((4, lambda tt=tt, cc=cc: emit_v_tt(1, xT1, v1, tt, cc)))

            y_sb0 = bpool.tile([P, NT, N], BF16, name="y_sb0", tag="y_sb")
            pending = None
            for h in range(H):
                att = emit_S_head(0, q8_0, k8_0, h, f0, h)
                if pending is not None:
                    emit_y_head(0, pending[0], v0, y_sb0, pending[1])
                pending = (att, h)
            emit_y_head(0, pending[0], v0, y_sb0, pending[1])
            while f0:
                f0.popleft()[1]()

            # heads(1): fillers are yT(0) transposes then proj(0)
            yT0 = bpool.tile([P, KT, T], BF16, name="yT0", tag="yT")
            f1: deque = deque()
            for qt in range(NT):
                f1.append((0, lambda qt=qt: emit_yT_qt(0, y_sb0, yT0, qt)))
            for tt in range(NT):
                for cc in range(CH):
                    f1.append(
                        (tt + 1, lambda tt=tt, cc=cc: emit_proj_tt(0, yT0, tt, cc))
                    )

            y_sb1 = bpool.tile([P, NT, N], BF16, name="y_sb1", tag="y_sb")
            pending = None
            for h in range(H):
                att = emit_S_head(1, q8_1, k8_1, h, f1, h)
                if pending is not None:
                    emit_y_head(1, pending[0], v1, y_sb1, pending[1])
                pending = (att, h)
            emit_y_head(1, pending[0], v1, y_sb1, pending[1])
            while f1:
                f1.popleft()[1]()

            # tail: yT(1) + proj(1)
            yT1 = bpool.tile([P, KT, T], BF16, name="yT1", tag="yT")
            for qt in range(NT):
                emit_yT_qt(1, y_sb1, yT1, qt)
            for tt in range(NT):
                for cc in range(CH):
                    emit_proj_tt(1, yT1, tt, cc)

    nc.compile()
    return nc


_CACHE: dict = {}


def _get_runner():
    """Build the Bass module once and wrap it in a cached jitted PJRT callable
    (mirrors concourse.bass2jax.run_bass_via_pjrt, but with a stable jit so
    repeated kernel() calls don't recompile)."""
    if "runner" in _CACHE:
        return _CACHE["runner"]

    import jax
    from jax.experimental.shard_map import shard_map
    from jax.sharding import Mesh, PartitionSpec

    from concourse import bass2jax

    nc = _build_nc()
    bass2jax.install_neuronx_cc_hook()

    partition_name = (
        nc.partition_id_tensor.name if nc.partition_id_tensor is not None else None
    )
    in_names: list[str] = []
    out_names: list[str] = []
    out_avals = []
    zero_outs: list[np.ndarray] = []
    for alloc in nc.m.functions[0].allocations:
        if not isinstance(alloc, mybir.MemoryLocationSet):
            continue
        name = alloc.memorylocations[0].name
        if alloc.kind == "ExternalInput":
            if name != partition_name:
                in_names.append(name)
        elif alloc.kind == "ExternalOutput":
            shape = tuple(alloc.tensor_shape)
            dtype = mybir.dt.np(alloc.dtype)
            out_names.append(name)
            out_avals.append(jax.core.ShapedArray(shape, dtype))
            zero_outs.append(np.zeros(shape, dtype))

    n_params = len(in_names)
    all_names = list(in_names) + list(out_names)
    if partition_name is not None:
        all_names.append(partition_name)
    all_names = tuple(all_names)
    donate = tuple(range(n_params, n_params + len(out_names)))

    def _body(*args):
        operands = list(args)
        if partition_name is not None:
            operands.append(bass2jax.partition_id_tensor())
        outs = bass2jax._bass_exec_p.bind(
            *operands,
            out_avals=tuple(out_avals),
            in_names=all_names,
            out_names=tuple(out_names),
            lowering_input_output_aliases=(),
            sim_require_finite=True,
            sim_require_nnan=True,
            nc=nc,
        )
        return tuple(outs)

    devices = jax.devices()[:NCORES]
    mesh = Mesh(np.asarray(devices), ("core",))
    n_io = n_params + len(out_names)
    sharded = jax.jit(
        shard_map(
            _body,
            mesh=mesh,
            in_specs=(PartitionSpec("core"),) * n_io,
            out_specs=(PartitionSpec("core"),) * len(out_names),
            check_rep=False,
        ),
        donate_argnums=donate,
        keep_unused=True,
    )

    def prep(per_core_inputs: list[dict]):
        """Stage concatenated inputs on the devices once; returns a thunk that
        launches one execution (fresh donated zero outputs each call)."""
        from jax.sharding import NamedSharding

        sh = NamedSharding(mesh, PartitionSpec("core"))
        concat_in = [
            jax.device_put(
                np.concatenate(
                    [np.asarray(m[name]) for m in per_core_inputs], axis=0
                ),
                sh,
            )
            for name in in_names
        ]
        mk_zeros = jax.jit(
            lambda: tuple(
                jax.numpy.zeros((NCORES * z.shape[0], *z.shape[1:]), z.dtype)
                for z in zero_outs
            ),
            out_shardings=(sh,) * len(zero_outs),
        )

        def launch():
            zs = mk_zeros()
            return sharded(*concat_in, *zs)

        return launch

    def run(per_core_inputs: list[dict]) -> list[np.ndarray]:
        """per_core_inputs: one dict per core keyed by in_names.
        Returns the per-core 'out' arrays."""
        out_arrs = prep(per_core_inputs)()
        (res,) = [np.asarray(a) for a in out_arrs]
        per_core_shape = out_avals[0].shape
        return list(res.reshape(NCORES, *per_core_shape))

    run.prep = prep
    _CACHE["runner"] = run
    return run


def _per_core_maps(inputs: dict) -> list[dict]:
    x = np.ascontiguousarray(np.asarray(inputs["x"], dtype=np.float32))
    wa = np.ascontiguousarray(np.asarray(inputs["W_attn"], dtype=np.float32))
    ba = np.ascontiguousarray(np.asarray(inputs["b_attn"], dtype=np.float32))
    wp = np.ascontiguousarray(np.asarray(inputs["W_proj"], dtype=np.float32))
    bp = np.ascontiguousarray(np.asarray(inputs["b_proj"], dtype=np.float32))
    x_slices = x.reshape(NCORES, BL * T, N)
    return [
        {"x": x_slices[i], "W_attn": wa, "b_attn": ba, "W_proj": wp, "b_proj": bp}
        for i in range(NCORES)
    ]


def kernel(**inputs) -> np.ndarray:
    run = _get_runner()
    outs = run(_per_core_maps(inputs))
    return np.concatenate(outs, axis=0).reshape(B, T, N).astype(np.float32)


if __name__ == "__main__":
    rng = np.random.default_rng(0)
    ins = {
        "x": rng.standard_normal((B, T, N), dtype=np.float32),
        "W_attn": (rng.standard_normal((N, 3 * N)) * 0.02).astype(np.float32),
        "b_attn": (rng.standard_normal((3 * N,)) * 0.02).astype(np.float32),
        "W_proj": (rng.standard_normal((N, N)) * 0.02).astype(np.float32),
        "b_proj": (rng.standard_normal((N,)) * 0.02).astype(np.float32),
    }
    out = kernel(**ins)
    print("kernel out:", out.shape, out.dtype, float(np.abs(out).mean()))
